# revision 10
# baseline (speedup 1.0000x reference)
"""AudioAttNet Trainium2 kernel (8-core SPMD).

Math (see reference):
  y  = leaky-conv-stack(x.T): 2048 -> 16 -> 8 -> 4 -> 2 -> 1 channels, k=3, pad=1
  logits = y @ Wl.T + bl          (Wl: [8192, 8192])
  att = softmax(logits)
  out = att @ x                   ([2048])

Sharding: sequence-sharded over 8 cores. Core k owns seq slice
[k*1024, (k+1)*1024).  It loads x.T columns for its slice (+8 halo) and the
1024 rows of Wl for its logit chunk (logit chunk == its own seq chunk).
Two tiny AllGathers: (1) conv outputs y (1024 floats/core), (2) per-core
softmax partition sums.  The big Wl matvec runs on the vector engine with Wl
in natural row-major layout using fused multiply+free-reduce
(scalar_tensor_tensor with accum_out), since a PE matmul would need Wl
transposed, which cannot be DMA'd efficiently in fp32.  Softmax uses a fixed
shift (logits are O(1)) so only the global sum Z needs communication; the
attention-weighted sum of x runs UNNORMALIZED from local exp(logits) and is
scaled by 1/Z at the very end, so the Z AllGather overlaps with it.
Host sums the 8 partial [2048]-vectors.
"""

import numpy as np

import concourse.bass as bass
import concourse.bacc as bacc
import concourse.tile as tile
import concourse.mybir as mybir
from concourse.bass_utils import run_bass_kernel_spmd

SEQ = 8192
DIM = 2048
NCORES = 8
CH = SEQ // NCORES          # 1024: per-core seq/logit chunk
HALO = 8
EXT = CH + 2 * HALO         # 1040 extended range
W = EXT + 2                 # 1042: buffer width with 1 zero pad col each side
CT = DIM // 128             # 16 channel tiles
WLW = 4096                  # Wl dma tile width (2 MB tiles)
F = 2048                    # matvec free-dim chunk per stt call
JT = CH // 128              # 8 j-tiles per core
NSUB = SEQ // F             # 4 accum slots per j-tile
NEG_SLOPE = 0.02
SHIFT = -10.0               # fixed softmax shift (logits are O(1))

f32 = mybir.dt.float32
Ax = mybir.AxisListType
Op = mybir.AluOpType
Act = mybir.ActivationFunctionType

CONV = [(DIM, 16), (16, 8), (8, 4), (4, 2), (2, 1)]
WOFF = [None, 0, 24, 36, 42]
NCHUNKS = [(0, 512), (512, 512), (1024, EXT - 1024)]

_CACHED_NC = None
LAST_RESULTS = None


def _build():
    nc = bacc.Bacc(
        "TRN2", target_bir_lowering=False, debug=False, num_devices=NCORES
    )
    xt_in = nc.dram_tensor("xt", [128, CT * EXT], f32, kind="ExternalInput")
    wl_in = nc.dram_tensor("wl", [CH, SEQ], f32, kind="ExternalInput")
    w1t_in = nc.dram_tensor("w1t", [128, CT * 48], f32, kind="ExternalInput")
    wst_in = nc.dram_tensor("wst", [16, 45], f32, kind="ExternalInput")
    bs_in = nc.dram_tensor("bs", [16, 5], f32, kind="ExternalInput")
    blc_in = nc.dram_tensor("blc", [128, JT], f32, kind="ExternalInput")
    mask_in = nc.dram_tensor("mask", [16, W], f32, kind="ExternalInput")
    out_d = nc.dram_tensor("out", [128, CT], f32, kind="ExternalOutput")

    rg = [list(range(NCORES))]

    with tile.TileContext(nc) as tc:
        with (
            tc.tile_pool(name="sb", bufs=1) as sb,
            tc.tile_pool(name="wlp", bufs=3) as wlp,
            tc.tile_pool(name="rot", bufs=2) as rot,
            tc.tile_pool(name="psp", bufs=2, space="PSUM") as psp,
            tc.tile_pool(name="ybp", bufs=4, space="PSUM") as ybp,
            tc.tile_pool(name="zpp", bufs=1, space="PSUM") as zpp,
            tc.tile_pool(name="dram", bufs=1, space="DRAM") as dram,
        ):
            # ---------------- small constant loads (SP ring) ----------------
            w1t = sb.tile([128, CT * 48], f32)
            nc.sync.dma_start(w1t[:], w1t_in[:])
            wst = sb.tile([16, 45], f32)
            nc.sync.dma_start(wst[:], wst_in[:])
            bs = sb.tile([16, 5], f32)
            nc.sync.dma_start(bs[:], bs_in[:])
            blc = sb.tile([128, JT], f32)
            nc.sync.dma_start(blc[:], blc_in[:])
            msk = sb.tile([16, W], f32)
            nc.sync.dma_start(msk[:], mask_in[:])

            # ---------------- x^T tiles: ONE big DMA, first on ACT ring ----
            xts = sb.tile([128, CT * W], f32)
            for ct in range(CT):
                nc.vector.memset(xts[:, ct * W : ct * W + 1], 0.0)
                nc.vector.memset(xts[:, ct * W + W - 1 : ct * W + W], 0.0)
            nc.scalar.dma_start(
                xts[:].rearrange("P (a c) -> P a c", a=CT)[:, :, 1 : W - 1],
                xt_in[:].rearrange("P (a c) -> P a c", a=CT),
            )

            # ---------------- Wl stream (ACT ring) + matvec (DVE) ----------
            y_b = sb.tile([128, SEQ], f32)
            pp = sb.tile([128, JT * NSUB], f32)
            for jt in range(JT):
                for half in range(SEQ // WLW):
                    wt = wlp.tile([128, WLW], f32, tag="wl")
                    nc.scalar.dma_start(
                        wt[:],
                        wl_in[
                            jt * 128 : (jt + 1) * 128,
                            half * WLW : (half + 1) * WLW,
                        ],
                    )
                    for s in range(WLW // F):
                        tcn = (half * WLW + s * F) // F
                        tr = rot.tile([128, F], f32, tag="tr")
                        nc.vector.scalar_tensor_tensor(
                            out=tr[:],
                            in0=wt[:, s * F : (s + 1) * F],
                            scalar=1.0,
                            in1=y_b[:, tcn * F : (tcn + 1) * F],
                            op0=Op.mult,
                            op1=Op.mult,
                            accum_out=pp[:, jt * NSUB + tcn : jt * NSUB + tcn + 1],
                        )

            # ---------------- conv stack (PE + DVE post) ----------------
            ybufs = [sb.tile([16, W], f32, tag=f"yb{i}") for i in range(2)]

            def ybuf(L):
                return ybufs[L % 2][0 : CONV[L][1], :]

            for i in range(2):
                nc.vector.memset(ybufs[i][:, 0:1], 0.0)
                nc.vector.memset(ybufs[i][:, W - 1 : W], 0.0)

            for L in range(5):
                cin, cout = CONV[L]
                for (n0, N) in NCHUNKS:
                    ps = psp.tile([16, 512], f32, tag="cps")
                    first = True
                    if L == 0:
                        for ct in range(CT):
                            for k in range(3):
                                nc.tensor.matmul(
                                    ps[0:cout, 0:N],
                                    w1t[:, ct * 48 + k * 16 : ct * 48 + k * 16 + 16],
                                    xts[:, ct * W + n0 + k : ct * W + n0 + k + N],
                                    start=first,
                                    stop=(ct == CT - 1 and k == 2),
                                )
                                first = False
                    else:
                        for k in range(3):
                            nc.tensor.matmul(
                                ps[0:cout, 0:N],
                                wst[0:cin, WOFF[L] + k * cout : WOFF[L] + (k + 1) * cout],
                                ybuf(L - 1)[:, n0 + k : n0 + k + N],
                                start=first,
                                stop=(k == 2),
                            )
                            first = False
                    z = rot.tile([16, 512], f32, tag="z")
                    nc.vector.tensor_scalar_add(
                        z[0:cout, 0:N], ps[0:cout, 0:N], bs[0:cout, L : L + 1]
                    )
                    yo = ybuf(L)[:, 1 + n0 : 1 + n0 + N]
                    nc.vector.scalar_tensor_tensor(
                        out=yo,
                        in0=z[0:cout, 0:N],
                        scalar=NEG_SLOPE,
                        in1=z[0:cout, 0:N],
                        op0=Op.mult,
                        op1=Op.max,
                    )
                    nc.vector.tensor_mul(
                        yo, yo, msk[0:cout, 1 + n0 : 1 + n0 + N]
                    )

            # ---------------- AllGather y, broadcast via PE ones ----------
            ycc_in = dram.tile([1, CH], f32)
            ycc_out = dram.tile([NCORES, CH], f32)
            nc.sync.dma_start(ycc_in[:], ybuf(4)[0:1, HALO + 1 : HALO + 1 + CH])
            nc.gpsimd.collective_compute(
                "AllGather",
                Op.bypass,
                replica_groups=rg,
                ins=[ycc_in[:].opt()],
                outs=[ycc_out[:].opt()],
            )
            nc.sync.dma_start(y_b[0:1, :], ycc_out[:].rearrange("a b -> (a b)"))
            ones = sb.tile([128, 1], f32)
            nc.vector.memset(ones[:], 1.0)
            for c0 in range(0, SEQ, 512):
                pb = ybp.tile([128, 512], f32, tag="ybc")
                nc.tensor.matmul(
                    pb[:], ones[:], y_b[0:1, c0 : c0 + 512],
                    start=True, stop=True,
                )
                nc.vector.tensor_copy(y_b[1:128, c0 : c0 + 512], pb[1:128, :])

            # ---------------- logits, fixed-shift exp ----------------
            logits = sb.tile([128, JT], f32)
            nc.vector.reduce_sum(
                logits[:],
                pp[:].rearrange("p (a b) -> p a b", a=JT),
                axis=Ax.X,
            )
            nc.vector.tensor_add(logits[:], logits[:], blc[:])

            shift = sb.tile([128, 1], f32)
            nc.vector.memset(shift[:], SHIFT)
            es = sb.tile([128, JT], f32)
            zp = sb.tile([128, 1], f32)
            nc.scalar.activation(
                es[:], logits[:], Act.Exp, bias=shift[:, 0:1], scale=1.0,
                accum_out=zp[:],
            )

            # ---- Z path: PE partition-sum -> AllGather -> 1/Z (overlapped
            # with the unnormalized weighted sum below)
            zps = zpp.tile([1, 1], f32, tag="zps")
            nc.tensor.matmul(zps[:], ones[:], zp[:], start=True, stop=True)
            zk = sb.tile([1, 1], f32)
            nc.vector.tensor_copy(zk[:], zps[:])
            stcc_in = dram.tile([1, 1], f32)
            stcc_out = dram.tile([1, NCORES], f32)
            nc.sync.dma_start(stcc_in[:], zk[:])
            nc.gpsimd.collective_compute(
                "AllGather",
                Op.bypass,
                replica_groups=rg,
                ins=[stcc_in[:].opt()],
                outs=[stcc_out[:].rearrange("o (a b) -> (o a) b", a=NCORES).opt()],
            )
            sta = sb.tile([1, NCORES], f32)
            nc.sync.dma_start(sta[:], stcc_out[:])
            zt = sb.tile([1, 1], f32)
            nc.vector.reduce_sum(zt[:], sta[:], axis=Ax.X)
            rz = sb.tile([1, 1], f32)
            nc.vector.reciprocal(rz[:], zt[:])
            rzb = sb.tile([128, 1], f32)
            nc.gpsimd.partition_broadcast(rzb[:], rz[:])

            # ---- es broadcast via DRAM bounce: es[pt, jt] is the
            # unnormalized weight for local t = jt*128 + pt; DRAM flat index
            # pt*JT + jt (contiguous per-partition writes).
            esd = dram.tile([1, CH], f32)
            nc.sync.dma_start(
                esd[:].rearrange("o (p j) -> (o p) j", j=JT), es[:]
            )
            esb = sb.tile([128, CH], f32)
            nc.sync.dma_start(esb[0:1, :], esd[:])
            nc.gpsimd.partition_broadcast(esb[:], esb[0:1, :])

            # ---- unnormalized weighted sum: outp[c] = sum_t es[t] xT[c, t]
            outp = sb.tile([128, CT], f32)
            esv = esb[:].rearrange("P (p j) -> P j p", j=JT)  # [128, JT, 128]
            for ct in range(CT):
                tr = rot.tile([128, F], f32, tag="tr")
                nc.vector.scalar_tensor_tensor(
                    out=tr[:, 0:CH].rearrange("P (a b) -> P a b", a=JT),
                    in0=xts[:, ct * W + HALO + 1 : ct * W + HALO + 1 + CH]
                    .rearrange("P (a b) -> P a b", a=JT),
                    scalar=1.0,
                    in1=esv,
                    op0=Op.mult,
                    op1=Op.mult,
                    accum_out=outp[:, ct : ct + 1],
                )
            # normalize by 1/Z and store
            nc.vector.tensor_scalar_mul(outp[:], outp[:], rzb[:, 0:1])
            nc.sync.dma_start(out_d[:], outp[:])

    nc.compile()
    return nc


def _get_nc():
    global _CACHED_NC
    if _CACHED_NC is None:
        _CACHED_NC = _build()
    return _CACHED_NC


def host_prep(**inputs):
    x = np.asarray(inputs["x"], np.float32)
    Wl = np.asarray(inputs["Wl"], np.float32)
    bl = np.asarray(inputs["bl"], np.float32)
    ws = [np.asarray(inputs[f"w{i}"], np.float32) for i in range(1, 6)]
    bss = [np.asarray(inputs[f"b{i}"], np.float32) for i in range(1, 6)]

    xT = np.ascontiguousarray(x.T)  # [DIM, SEQ]

    # packed conv1 weights: w1t[c128, ct*48 + k*16 + o] = w1[o, ct*128+c128, k]
    w1r = np.transpose(ws[0], (1, 2, 0))  # [c, k, o]
    w1t = np.ascontiguousarray(
        w1r.reshape(CT, 128, 48).transpose(1, 0, 2).reshape(128, CT * 48)
    )
    wst = np.zeros((16, 45), np.float32)
    for L in range(1, 5):
        w = np.transpose(ws[L], (1, 2, 0))  # [cin, k, o]
        wst[0 : w.shape[0], WOFF[L] : WOFF[L] + w.shape[1] * w.shape[2]] = (
            w.reshape(w.shape[0], -1)
        )
    bs = np.zeros((16, 5), np.float32)
    for L in range(5):
        bs[0 : bss[L].shape[0], L] = bss[L]

    in_maps = []
    for k in range(NCORES):
        s0 = k * CH
        lo, hi = s0 - HALO, s0 + CH + HALO
        xt_k = np.zeros((DIM, EXT), np.float32)
        glo, ghi = max(lo, 0), min(hi, SEQ)
        xt_k[:, glo - lo : ghi - lo] = xT[:, glo:ghi]
        # device layout: [128, ct, EXT]
        xt_k = np.ascontiguousarray(
            xt_k.reshape(CT, 128, EXT).transpose(1, 0, 2).reshape(128, CT * EXT)
        )
        mask_k = np.zeros((16, W), np.float32)
        tt = np.arange(lo, hi)
        mask_k[:, 1 : W - 1] = ((tt >= 0) & (tt < SEQ)).astype(np.float32)[
            None, :
        ]
        blc_k = np.ascontiguousarray(bl[s0 : s0 + CH].reshape(JT, 128).T)
        wl_k = np.ascontiguousarray(Wl[s0 : s0 + CH, :])
        in_maps.append(
            {
                "xt": xt_k,
                "wl": wl_k,
                "w1t": w1t,
                "wst": wst,
                "bs": bs,
                "blc": blc_k,
                "mask": mask_k,
            }
        )
    return in_maps


def kernel(**inputs):
    global LAST_RESULTS
    in_maps = host_prep(**inputs)
    nc = _get_nc()
    res = run_bass_kernel_spmd(nc, in_maps, core_ids=list(range(NCORES)))
    LAST_RESULTS = res

    total = np.zeros((128, CT), np.float64)
    for r in res.results:
        total += r["out"]
    return np.ascontiguousarray(total.T.reshape(DIM)).astype(np.float32)


# revision 29
# speedup vs baseline: 4.3582x; 4.3582x over previous
"""AudioAttNet Trainium2 kernel (8-core SPMD).

Math (see reference):
  y  = leaky-conv-stack(x.T): 2048 -> 16 -> 8 -> 4 -> 2 -> 1 channels, k=3, pad=1
  logits = y @ Wl.T + bl          (Wl: [8192, 8192])
  att = softmax(logits)
  out = att @ x                   ([2048])

Sharding: sequence-sharded over 8 cores. Core k owns seq slice
[k*1024, (k+1)*1024).  It loads x.T columns for its slice (+8 halo) and the
1024 rows of Wl for its logit chunk (logit chunk == its own seq chunk).
ONE tiny AllGather (conv outputs y).  The big Wl matvec runs on the vector
engine with Wl in natural row-major layout using fused multiply+free-reduce
(scalar_tensor_tensor with accum_out) — a PE matmul would need Wl
transposed, which cannot be DMA'd efficiently in fp32.  Conv1 runs on PE
with the x.T window as the STATIONARY operand (fp32 matmuls stream at ~4
cycles/col, so streaming the wide x.T per tap would be ~150us; streaming
the 48-wide packed w1 instead is ~30us).  Softmax uses a fixed shift
(logits are O(1)); the attention-weighted sum runs per-j-tile, pipelined
with the Wl stream, on UNNORMALIZED exp(logits); the global softmax
denominator is summed on the host across cores (each core also returns its
local sum of exponentials).
"""

import numpy as np

import concourse.bass as bass
import concourse.bacc as bacc
import concourse.tile as tile
import concourse.mybir as mybir
from concourse.tile import add_dep_helper
from concourse.bass_utils import run_bass_kernel_spmd

SEQ = 8192
DIM = 2048
NCORES = 8
CH = SEQ // NCORES          # 1024: per-core seq/logit chunk
HALO = 8
EXT = CH + 2 * HALO         # 1040 extended range
W = EXT + 2                 # 1042: buffer width, 1 zero pad col each side
CT = DIM // 128             # 16 channel tiles
F = 2048                    # matvec free-dim chunk per stt call / wl tile
JT = CH // 128              # 8 j-tiles per core
NSUB = SEQ // F             # 4 t-chunks per j-tile
NTT = (W + 127) // 128      # 9 t-tiles for the conv1 z computation
NEG_SLOPE = 0.02
SHIFT = -10.0               # fixed softmax shift (logits are O(1))

f32 = mybir.dt.float32
Ax = mybir.AxisListType
Op = mybir.AluOpType
Act = mybir.ActivationFunctionType

CONV = [(DIM, 16), (16, 8), (8, 4), (4, 2), (2, 1)]
WOFF = [None, 0, 24, 36, 42]
NCHUNKS = [(0, 512), (512, 512), (1024, EXT - 1024)]

_CACHED_NC = None
LAST_RESULTS = None


def _build(single=False):
    # single=True: 1-core variant with the collective replaced by local DMA
    # copies — numerically wrong across cores, used only for TimelineSim.
    nc = bacc.Bacc(
        "TRN2", target_bir_lowering=False, debug=False,
        num_devices=1 if single else NCORES,
    )
    xt_in = nc.dram_tensor("xt", [128, CT * EXT], f32, kind="ExternalInput")
    wl_in = nc.dram_tensor("wl", [CH, SEQ], f32, kind="ExternalInput")
    w1t_in = nc.dram_tensor("w1t", [128, CT * 48], f32, kind="ExternalInput")
    wst_in = nc.dram_tensor("wst", [17, 45], f32, kind="ExternalInput")
    bs_in = nc.dram_tensor("bs", [16, 1], f32, kind="ExternalInput")
    blc_in = nc.dram_tensor("blc", [128, JT], f32, kind="ExternalInput")
    mask_in = nc.dram_tensor("mask", [16, W], f32, kind="ExternalInput")
    out_d = nc.dram_tensor("out", [128, CT * JT], f32, kind="ExternalOutput")
    z_d = nc.dram_tensor("zout", [1, 1], f32, kind="ExternalOutput")

    rg = [list(range(NCORES))]

    with tile.TileContext(nc) as tc:
        with (
            tc.tile_pool(name="sb", bufs=1) as sb,
            tc.tile_pool(name="wlp", bufs=7) as wlp,
            tc.tile_pool(name="bigp", bufs=4) as bigp,
            tc.tile_pool(name="rot", bufs=2) as rot,
            tc.tile_pool(name="psp", bufs=2, space="PSUM") as psp,
            tc.tile_pool(name="zpl", bufs=2, space="PSUM") as zpl,
            tc.tile_pool(name="ztp", bufs=3, space="PSUM") as ztp,
            tc.tile_pool(name="zpp", bufs=1, space="PSUM") as zpp,
            tc.tile_pool(name="dram", bufs=1, space="DRAM") as dram,
        ):
            # ---------------- small constant loads (SP ring) ----------------
            w1t = sb.tile([128, CT * 48], f32)
            nc.sync.dma_start(w1t[:], w1t_in[:])
            wst = sb.tile([17, 45], f32)
            nc.sync.dma_start(wst[:], wst_in[:])
            bs = sb.tile([16, 1], f32)
            nc.sync.dma_start(bs[:], bs_in[:])
            blc = sb.tile([128, JT], f32)
            nc.sync.dma_start(blc[:], blc_in[:])
            msk = sb.tile([16, W], f32)
            nc.sync.dma_start(msk[:], mask_in[:])
            ident = sb.tile([128, 128], f32)
            idn = nc.inline_tensor(np.eye(128, dtype=np.float32))
            nc.sync.dma_start(ident[:], idn[:])
            ones_row_d = nc.inline_tensor(np.ones((1, W), dtype=np.float32))

            # ---------------- x^T tiles: ONE big DMA, first on ACT ring ----
            xts = sb.tile([128, CT * W], f32)
            for ct in range(CT):
                nc.vector.memset(xts[:, ct * W : ct * W + 1], 0.0)
                nc.vector.memset(xts[:, ct * W + W - 1 : ct * W + W], 0.0)
            nc.scalar.dma_start(
                xts[:].rearrange("P (a c) -> P a c", a=CT)[:, :, 1 : W - 1],
                xt_in[:].rearrange("P (a c) -> P a c", a=CT),
            )

            # ---------------- Wl stream (ACT ring): DMA issuance only ------
            # stt consumers are traced AFTER the y broadcasts (Tile deps
            # follow trace order).  Order: (tcn=0, jt=0..7) then jt-major so
            # each j-tile finishes early and its softmax/weighted-sum tail
            # work pipelines with the stream.
            wl_order = [(0, jt) for jt in range(JT)] + [
                (tcn, jt) for jt in range(JT) for tcn in range(1, NSUB)
            ]
            wtiles = {}
            for (tcn, jt) in wl_order:
                wt = wlp.tile([128, F], f32, tag="wl", bufs=7)
                nc.scalar.dma_start(
                    wt[:],
                    wl_in[jt * 128 : (jt + 1) * 128, tcn * F : (tcn + 1) * F],
                )
                wtiles[(tcn, jt)] = wt

            # ---------------- conv1 via stationary-x z-trick ----------------
            #   z[t, (k, o)] = sum_c x[t, c] w1[o, c, k]
            # transpose each k-slice of z (PE), then y1 = sum of the three
            # free-dim-shifted slices.
            zT0 = bigp.tile([16, NTT * 128], f32, tag="big", bufs=4)
            zT1 = bigp.tile([16, NTT * 128], f32, tag="big", bufs=4)
            zT2 = bigp.tile([16, NTT * 128], f32, tag="big", bufs=4)
            zTs = [zT0, zT1, zT2]
            for tt in range(NTT):
                m0 = tt * 128
                M = min(128, W - m0)
                zp_ = zpl.tile([128, 48], f32, tag="zp")
                for ct in range(CT):
                    nc.tensor.matmul(
                        zp_[0:M, :],
                        xts[:, ct * W + m0 : ct * W + m0 + M],
                        w1t[:, ct * 48 : (ct + 1) * 48],
                        start=(ct == 0),
                        stop=(ct == CT - 1),
                    )
                zsb = rot.tile([128, 48], f32, tag="zsb")
                nc.vector.tensor_copy(zsb[0:M, :], zp_[0:M, :])
                for kk in range(3):
                    pt = ztp.tile([16, 128], f32, tag="zt")
                    nc.tensor.transpose(
                        pt[0:16, 0:M],
                        zsb[0:M, kk * 16 : (kk + 1) * 16],
                        ident[0:M, 0:M],
                    )
                    nc.vector.tensor_copy(
                        zTs[kk][:, m0 : m0 + M], pt[0:16, 0:M]
                    )

            # ybufs [17, W]: row CONV[L][1] holds ONES (bias row for the
            # next layer's augmented matmul)
            yb0 = sb.tile([17, W], f32)
            yb1 = sb.tile([17, W], f32)
            ybufs = [yb0, yb1]

            def ybuf(L):
                return ybufs[L % 2]

            for i in range(2):
                nc.vector.memset(ybufs[i][:, 0:1], 0.0)
                nc.vector.memset(ybufs[i][:, W - 1 : W], 0.0)

            # y1[:, m] = z0[m-1] + z1[m] + z2[m+1] + b1, m in [1, W-1)
            NV = W - 2
            z1t = rot.tile([16, W], f32, tag="z1t")
            nc.vector.tensor_add(z1t[:, 0:NV], zT0[:, 0:NV], zT1[:, 1 : 1 + NV])
            nc.vector.tensor_add(z1t[:, 0:NV], z1t[:, 0:NV], zT2[:, 2 : 2 + NV])
            nc.vector.tensor_scalar_add(z1t[:, 0:NV], z1t[:, 0:NV], bs[:, 0:1])
            y1w = ybuf(0)
            nc.vector.scalar_tensor_tensor(
                out=y1w[0:16, 1 : 1 + NV],
                in0=z1t[0:16, 0:NV],
                scalar=NEG_SLOPE,
                in1=z1t[0:16, 0:NV],
                op0=Op.mult,
                op1=Op.max,
            )
            nc.vector.tensor_mul(
                y1w[0:16, 1 : 1 + NV], y1w[0:16, 1 : 1 + NV],
                msk[0:16, 1 : 1 + NV],
            )
            nc.sync.dma_start(y1w[16:17, :], ones_row_d[:])  # bias row, conv2

            # ---------------- convs 2-5 (PE, bias via augmented row) -------
            for L in range(1, 5):
                cin, cout = CONV[L]
                yprev = ybuf(L - 1)
                ycur = ybuf(L)
                for (n0, N) in NCHUNKS:
                    ps = psp.tile([16, 512], f32, tag="cps")
                    for k in range(3):
                        kin = cin + 1 if k == 1 else cin  # bias row on k=1
                        nc.tensor.matmul(
                            ps[0:cout, 0:N],
                            wst[0:kin, WOFF[L] + k * cout : WOFF[L] + (k + 1) * cout],
                            yprev[0:kin, n0 + k : n0 + k + N],
                            start=(k == 0),
                            stop=(k == 2),
                        )
                    # PSUM -> SBUF on ACT, then leaky on DVE
                    zc = rot.tile([16, 512], f32, tag="zc", name=f"zc{L}_{n0}")
                    nc.scalar.copy(zc[0:cout, 0:N], ps[0:cout, 0:N])
                    yo = ycur[0:cout, 1 + n0 : 1 + n0 + N]
                    nc.vector.scalar_tensor_tensor(
                        out=yo,
                        in0=zc[0:cout, 0:N],
                        scalar=NEG_SLOPE,
                        in1=zc[0:cout, 0:N],
                        op0=Op.mult,
                        op1=Op.max,
                    )
                nc.vector.tensor_mul(
                    ycur[0:cout, 1 : 1 + NV], ycur[0:cout, 1 : 1 + NV],
                    msk[0:cout, 1 : 1 + NV],
                )
                if L < 4:
                    nc.sync.dma_start(ycur[cout : cout + 1, :], ones_row_d[:])

            # ---------------- AllGather y, per-chunk broadcast -------------
            ycc_in = dram.tile([1, CH], f32)
            ycc_out = dram.tile([NCORES, CH], f32)
            nc.sync.dma_start(ycc_in[:], ybuf(4)[0:1, HALO + 1 : HALO + 1 + CH])
            if single:
                for r in range(NCORES):
                    nc.sync.dma_start(ycc_out[r : r + 1, :], ycc_in[:])
            else:
                nc.gpsimd.collective_compute(
                    "AllGather",
                    Op.bypass,
                    replica_groups=rg,
                    ins=[ycc_in[:].opt()],
                    outs=[ycc_out[:].opt()],
                )
            ycc_flat = ycc_out[:].rearrange("a b -> (a b)")
            ychs = []
            for tcn in range(NSUB):
                ych = bigp.tile([128, F], f32, tag="big", bufs=4)
                nc.sync.dma_start(ych[0:1, :], ycc_flat[tcn * F : (tcn + 1) * F])
                nc.gpsimd.partition_broadcast(ych[:], ych[0:1, :])
                ychs.append(ych)

            ones = sb.tile([128, 1], f32)
            nc.vector.memset(ones[:], 1.0)
            shift = sb.tile([128, 1], f32)
            nc.vector.memset(shift[:], SHIFT)

            # ---------------- matvec + per-j-tile softmax/weighted sum -----
            pp = sb.tile([128, JT * NSUB], f32)
            es = sb.tile([128, JT], f32)
            zp = sb.tile([128, JT], f32)
            opp = sb.tile([128, CT * JT], f32)

            def mv(tcn, jt):
                tr = rot.tile([128, F], f32, tag="tr", name=f"tr_{tcn}_{jt}")
                return nc.vector.scalar_tensor_tensor(
                    out=tr[:],
                    in0=wtiles[(tcn, jt)][:],
                    scalar=1.0,
                    in1=ychs[tcn][:],
                    op0=Op.mult,
                    op1=Op.mult,
                    accum_out=pp[:, jt * NSUB + tcn : jt * NSUB + tcn + 1],
                )

            def jt_tail(jt):
                # logits_jt -> exp -> transpose -> broadcast -> weighted sum
                lg = rot.tile([128, 1], f32, tag="lg", name=f"lg{jt}", bufs=2)
                nc.vector.reduce_sum(
                    lg[:],
                    pp[:, jt * NSUB : (jt + 1) * NSUB],
                    axis=Ax.X,
                )
                nc.vector.tensor_add(lg[:], lg[:], blc[:, jt : jt + 1])
                nc.scalar.activation(
                    es[:, jt : jt + 1], lg[:], Act.Exp, bias=shift[:, 0:1],
                    scale=1.0, accum_out=zp[:, jt : jt + 1],
                )
                ept = ztp.tile([16, 128], f32, tag="zt", name=f"ept{jt}", bufs=3)
                nc.tensor.transpose(
                    ept[0:1, :], es[:, jt : jt + 1], ident[:]
                )
                erow = rot.tile([128, 128], f32, tag="erow", name=f"erow{jt}",
                                bufs=2)
                nc.vector.tensor_copy(erow[0:1, :], ept[0:1, :])
                nc.gpsimd.partition_broadcast(erow[:], erow[0:1, :])
                last = None
                for ct in range(CT):
                    trw = rot.tile([128, 128], f32, tag="trw",
                                   name=f"trw{jt}_{ct}", bufs=2)
                    base = ct * W + HALO + 1 + jt * 128
                    last = nc.vector.scalar_tensor_tensor(
                        out=trw[:],
                        in0=xts[:, base : base + 128],
                        scalar=1.0,
                        in1=erow[:],
                        op0=Op.mult,
                        op1=Op.mult,
                        accum_out=opp[:, ct * JT + jt : ct * JT + jt + 1],
                    )
                return last

            for jt in range(JT):
                mv(0, jt)
            prev_tail = None
            for jt in range(JT):
                for tcn in range(1, NSUB):
                    mvi = mv(tcn, jt)
                    if prev_tail is not None and tcn == 1:
                        # keep each j-tile's softmax/weighted-sum tail ahead
                        # of the next j-tile's matvec in the DVE queue, so
                        # tails overlap the Wl stream instead of piling up
                        # at the end
                        add_dep_helper(
                            mvi.ins, prev_tail.ins,
                            reason="interleave jt tail with stream",
                        )
                prev_tail = jt_tail(jt)

            # ---------------- outputs ----------------
            nc.sync.dma_start(out_d[:], opp[:])
            # local softmax denominator: sum zp over free then partitions
            zpr = sb.tile([128, 1], f32)
            nc.vector.reduce_sum(zpr[:], zp[:], axis=Ax.X)
            zps = zpp.tile([1, 1], f32, tag="zps")
            nc.tensor.matmul(zps[:], ones[:], zpr[:], start=True, stop=True)
            zk = sb.tile([1, 1], f32)
            nc.vector.tensor_copy(zk[:], zps[:])
            nc.sync.dma_start(z_d[:], zk[:])

    nc.compile()
    return nc


def _get_nc():
    global _CACHED_NC
    if _CACHED_NC is None:
        _CACHED_NC = _build()
    return _CACHED_NC


def host_prep(**inputs):
    x = np.asarray(inputs["x"], np.float32)
    Wl = np.asarray(inputs["Wl"], np.float32)
    bl = np.asarray(inputs["bl"], np.float32)
    ws = [np.asarray(inputs[f"w{i}"], np.float32) for i in range(1, 6)]
    bss = [np.asarray(inputs[f"b{i}"], np.float32) for i in range(1, 6)]

    xT = np.ascontiguousarray(x.T)  # [DIM, SEQ]

    # packed conv1 weights: w1t[c128, ct*48 + k*16 + o] = w1[o, ct*128+c128, k]
    w1r = np.transpose(ws[0], (1, 2, 0))  # [c, k, o]
    w1t = np.ascontiguousarray(
        w1r.reshape(CT, 128, 48).transpose(1, 0, 2).reshape(128, CT * 48)
    )
    # packed conv2-5 weights + bias row (k=1 slice, row cin)
    wst = np.zeros((17, 45), np.float32)
    for L in range(1, 5):
        cin, cout = CONV[L]
        w = np.transpose(ws[L], (1, 2, 0))  # [cin, k, cout]
        wst[0:cin, WOFF[L] : WOFF[L] + 3 * cout] = w.reshape(cin, -1)
        wst[cin, WOFF[L] + cout : WOFF[L] + 2 * cout] = bss[L]
    bs = np.zeros((16, 1), np.float32)
    bs[:, 0] = bss[0]

    in_maps = []
    for k in range(NCORES):
        s0 = k * CH
        lo, hi = s0 - HALO, s0 + CH + HALO
        xt_k = np.zeros((DIM, EXT), np.float32)
        glo, ghi = max(lo, 0), min(hi, SEQ)
        xt_k[:, glo - lo : ghi - lo] = xT[:, glo:ghi]
        xt_k = np.ascontiguousarray(
            xt_k.reshape(CT, 128, EXT).transpose(1, 0, 2).reshape(128, CT * EXT)
        )
        mask_k = np.zeros((16, W), np.float32)
        tt = np.arange(lo, hi)
        mask_k[:, 1 : W - 1] = ((tt >= 0) & (tt < SEQ)).astype(np.float32)[
            None, :
        ]
        blc_k = np.ascontiguousarray(bl[s0 : s0 + CH].reshape(JT, 128).T)
        wl_k = np.ascontiguousarray(Wl[s0 : s0 + CH, :])
        in_maps.append(
            {
                "xt": xt_k,
                "wl": wl_k,
                "w1t": w1t,
                "wst": wst,
                "bs": bs,
                "blc": blc_k,
                "mask": mask_k,
            }
        )
    return in_maps


def kernel(**inputs):
    global LAST_RESULTS
    in_maps = host_prep(**inputs)
    nc = _get_nc()
    res = run_bass_kernel_spmd(nc, in_maps, core_ids=list(range(NCORES)))
    LAST_RESULTS = res

    total = np.zeros((128, CT * JT), np.float64)
    zsum = 0.0
    for r in res.results:
        total += r["out"]
        zsum += float(r["zout"][0, 0])
    # opp[p, ct*JT + jt] = sum over local t of es * xT; partials summed over
    # cores and j-tiles, then normalized by the global sum of exponentials.
    tot = total.reshape(128, CT, JT).sum(axis=2) / zsum
    return np.ascontiguousarray(tot.T.reshape(DIM)).astype(np.float32)


# revision 35
# speedup vs baseline: 5.2137x; 1.1963x over previous
"""AudioAttNet Trainium2 kernel (8-core SPMD).

Math (see reference):
  y  = leaky-conv-stack(x.T): 2048 -> 16 -> 8 -> 4 -> 2 -> 1 channels, k=3, pad=1
  logits = y @ Wl.T + bl          (Wl: [8192, 8192])
  att = softmax(logits)
  out = att @ x                   ([2048])

Sharding: sequence-sharded over 8 cores. Core k owns seq slice
[k*1024, (k+1)*1024).  It loads x.T columns for its slice (+8 halo) and the
1024 rows of Wl for its logit chunk (logit chunk == its own seq chunk).
ONE tiny AllGather (conv outputs y).  The big Wl matvec runs on the vector
engine with Wl in natural row-major layout using fused multiply+free-reduce
(scalar_tensor_tensor with accum_out) — a PE matmul would need Wl
transposed, which cannot be DMA'd efficiently in fp32.  Conv1 runs on PE
with the x.T window as the STATIONARY operand (fp32 matmuls stream at ~4
cycles/col, so streaming the wide x.T per tap would be ~150us; streaming
the 48-wide packed w1 instead is ~30us).  Softmax uses a fixed shift
(logits are O(1)); the attention-weighted sum runs per-j-tile, pipelined
with the Wl stream, on UNNORMALIZED exp(logits); the global softmax
denominator is summed on the host across cores (each core also returns its
local sum of exponentials).
"""

import numpy as np

import concourse.bass as bass
import concourse.bacc as bacc
import concourse.tile as tile
import concourse.mybir as mybir
from concourse.tile import add_dep_helper
from concourse.bass_utils import run_bass_kernel_spmd

SEQ = 8192
DIM = 2048
NCORES = 8
CH = SEQ // NCORES          # 1024: per-core seq/logit chunk
HALO = 8
EXT = CH + 2 * HALO         # 1040 extended range
W = EXT + 2                 # 1042: buffer width, 1 zero pad col each side
CT = DIM // 128             # 16 channel tiles
F = 2048                    # matvec free-dim chunk per stt call / wl tile
JT = CH // 128              # 8 j-tiles per core
NSUB = SEQ // F             # 4 t-chunks per j-tile
NTT = (W + 127) // 128      # 9 t-tiles for the conv1 z computation
NEG_SLOPE = 0.02
SHIFT = -10.0               # fixed softmax shift (logits are O(1))

f32 = mybir.dt.float32
Ax = mybir.AxisListType
Op = mybir.AluOpType
Act = mybir.ActivationFunctionType

CONV = [(DIM, 16), (16, 8), (8, 4), (4, 2), (2, 1)]
WOFF = [None, 0, 24, 36, 42]
NCHUNKS = [(0, 512), (512, 512), (1024, EXT - 1024)]

_CACHED_NC = None
LAST_RESULTS = None


def _build(single=False):
    # single=True: 1-core variant with the collective replaced by local DMA
    # copies — numerically wrong across cores, used only for TimelineSim.
    nc = bacc.Bacc(
        "TRN2", target_bir_lowering=False, debug=False,
        num_devices=1 if single else NCORES,
    )
    xt_in = nc.dram_tensor("xt", [128, CT * EXT], f32, kind="ExternalInput")
    wl_in = nc.dram_tensor("wl", [CH, SEQ], f32, kind="ExternalInput")
    w1t_in = nc.dram_tensor("w1t", [128, CT * 48], f32, kind="ExternalInput")
    wst_in = nc.dram_tensor("wst", [17, 45], f32, kind="ExternalInput")
    bs_in = nc.dram_tensor("bs", [16, 1], f32, kind="ExternalInput")
    blc_in = nc.dram_tensor("blc", [128, JT], f32, kind="ExternalInput")
    mask_in = nc.dram_tensor("mask", [16, W], f32, kind="ExternalInput")
    out_d = nc.dram_tensor("out", [128, CT * JT], f32, kind="ExternalOutput")
    z_d = nc.dram_tensor("zout", [1, 1], f32, kind="ExternalOutput")

    rg = [list(range(NCORES))]

    with tile.TileContext(nc) as tc:
        with (
            tc.tile_pool(name="sb", bufs=1) as sb,
            tc.tile_pool(name="wlp", bufs=7) as wlp,
            tc.tile_pool(name="bigp", bufs=4) as bigp,
            tc.tile_pool(name="rot", bufs=2) as rot,
            tc.tile_pool(name="psp", bufs=2, space="PSUM") as psp,
            tc.tile_pool(name="zpl", bufs=2, space="PSUM") as zpl,
            tc.tile_pool(name="ztp", bufs=3, space="PSUM") as ztp,
            tc.tile_pool(name="zpp", bufs=1, space="PSUM") as zpp,
            tc.tile_pool(name="dram", bufs=1, space="DRAM") as dram,
        ):
            # ---------------- small constant loads (SP ring) ----------------
            w1t = sb.tile([128, CT * 48], f32)
            nc.sync.dma_start(w1t[:], w1t_in[:])
            wst = sb.tile([17, 45], f32)
            nc.sync.dma_start(wst[:], wst_in[:])
            bs = sb.tile([16, 1], f32)
            nc.sync.dma_start(bs[:], bs_in[:])
            blc = sb.tile([128, JT], f32)
            nc.sync.dma_start(blc[:], blc_in[:])
            msk = sb.tile([16, W], f32)
            nc.sync.dma_start(msk[:], mask_in[:])
            ident = sb.tile([128, 128], f32)
            idn = nc.inline_tensor(np.eye(128, dtype=np.float32))
            nc.sync.dma_start(ident[:], idn[:])
            ones_row_d = nc.inline_tensor(np.ones((1, W), dtype=np.float32))

            # ---------------- x^T tiles: ONE big DMA, first on ACT ring ----
            xts = sb.tile([128, CT * W], f32)
            for ct in range(CT):
                nc.vector.memset(xts[:, ct * W : ct * W + 1], 0.0)
                nc.vector.memset(xts[:, ct * W + W - 1 : ct * W + W], 0.0)
            nc.scalar.dma_start(
                xts[:].rearrange("P (a c) -> P a c", a=CT)[:, :, 1 : W - 1],
                xt_in[:].rearrange("P (a c) -> P a c", a=CT),
            )

            # ---------------- Wl stream (ACT ring): DMA issuance only ------
            # stt consumers are traced AFTER the y broadcasts (Tile deps
            # follow trace order).  Order: (tcn=0, jt=0..7) then jt-major so
            # each j-tile finishes early and its softmax/weighted-sum tail
            # work pipelines with the stream.
            wl_order = [(0, jt) for jt in range(JT)] + [
                (tcn, jt) for jt in range(JT) for tcn in range(1, NSUB)
            ]
            wtiles = {}
            for (tcn, jt) in wl_order:
                wt = wlp.tile([128, F], f32, tag="wl", bufs=7)
                nc.scalar.dma_start(
                    wt[:],
                    wl_in[jt * 128 : (jt + 1) * 128, tcn * F : (tcn + 1) * F],
                )
                wtiles[(tcn, jt)] = wt

            # ---------------- conv1 via stationary-x z-trick ----------------
            #   z[t, (k, o)] = sum_c x[t, c] w1[o, c, k]
            # transpose each k-slice of z (PE), then y1 = sum of the three
            # free-dim-shifted slices.
            zT0 = bigp.tile([16, NTT * 128], f32, tag="big", bufs=4)
            zT1 = bigp.tile([16, NTT * 128], f32, tag="big", bufs=4)
            zT2 = bigp.tile([16, NTT * 128], f32, tag="big", bufs=4)
            zTs = [zT0, zT1, zT2]
            for tt in range(NTT):
                m0 = tt * 128
                M = min(128, W - m0)
                zp_ = zpl.tile([128, 48], f32, tag="zp")
                for ct in range(CT):
                    nc.tensor.matmul(
                        zp_[0:M, :],
                        xts[:, ct * W + m0 : ct * W + m0 + M],
                        w1t[:, ct * 48 : (ct + 1) * 48],
                        start=(ct == 0),
                        stop=(ct == CT - 1),
                    )
                zsb = rot.tile([128, 48], f32, tag="zsb")
                nc.vector.tensor_copy(zsb[0:M, :], zp_[0:M, :])
                for kk in range(3):
                    pt = ztp.tile([16, 128], f32, tag="zt")
                    nc.tensor.transpose(
                        pt[0:16, 0:M],
                        zsb[0:M, kk * 16 : (kk + 1) * 16],
                        ident[0:M, 0:M],
                    )
                    nc.vector.tensor_copy(
                        zTs[kk][:, m0 : m0 + M], pt[0:16, 0:M]
                    )

            # ybufs [17, W]: row CONV[L][1] holds ONES (bias row for the
            # next layer's augmented matmul)
            yb0 = sb.tile([17, W], f32)
            yb1 = sb.tile([17, W], f32)
            ybufs = [yb0, yb1]

            def ybuf(L):
                return ybufs[L % 2]

            for i in range(2):
                nc.vector.memset(ybufs[i][:, 0:1], 0.0)
                nc.vector.memset(ybufs[i][:, W - 1 : W], 0.0)

            # y1[:, m] = z0[m-1] + z1[m] + z2[m+1] + b1, m in [1, W-1)
            NV = W - 2
            z1t = rot.tile([16, W], f32, tag="z1t")
            nc.vector.tensor_add(z1t[:, 0:NV], zT0[:, 0:NV], zT1[:, 1 : 1 + NV])
            nc.vector.tensor_add(z1t[:, 0:NV], z1t[:, 0:NV], zT2[:, 2 : 2 + NV])
            nc.vector.tensor_scalar_add(z1t[:, 0:NV], z1t[:, 0:NV], bs[:, 0:1])
            y1w = ybuf(0)
            nc.vector.scalar_tensor_tensor(
                out=y1w[0:16, 1 : 1 + NV],
                in0=z1t[0:16, 0:NV],
                scalar=NEG_SLOPE,
                in1=z1t[0:16, 0:NV],
                op0=Op.mult,
                op1=Op.max,
            )
            # mask is 1.0 except in the 8-col halo edges — multiply only there
            for e0 in (1, W - 1 - HALO):
                nc.vector.tensor_mul(
                    y1w[0:16, e0 : e0 + HALO], y1w[0:16, e0 : e0 + HALO],
                    msk[0:16, e0 : e0 + HALO],
                )
            nc.sync.dma_start(y1w[16:17, :], ones_row_d[:])  # bias row, conv2

            # ---------------- convs 2-5 (PE, bias via augmented row) -------
            for L in range(1, 5):
                cin, cout = CONV[L]
                yprev = ybuf(L - 1)
                ycur = ybuf(L)
                for (n0, N) in NCHUNKS:
                    ps = psp.tile([16, 512], f32, tag="cps")
                    for k in range(3):
                        kin = cin + 1 if k == 1 else cin  # bias row on k=1
                        nc.tensor.matmul(
                            ps[0:cout, 0:N],
                            wst[0:kin, WOFF[L] + k * cout : WOFF[L] + (k + 1) * cout],
                            yprev[0:kin, n0 + k : n0 + k + N],
                            start=(k == 0),
                            stop=(k == 2),
                        )
                    # PSUM -> SBUF on ACT, then leaky on DVE
                    zc = rot.tile([16, 512], f32, tag="zc", name=f"zc{L}_{n0}")
                    nc.scalar.copy(zc[0:cout, 0:N], ps[0:cout, 0:N])
                    yo = ycur[0:cout, 1 + n0 : 1 + n0 + N]
                    nc.vector.scalar_tensor_tensor(
                        out=yo,
                        in0=zc[0:cout, 0:N],
                        scalar=NEG_SLOPE,
                        in1=zc[0:cout, 0:N],
                        op0=Op.mult,
                        op1=Op.max,
                    )
                for e0 in (1, W - 1 - HALO):
                    nc.vector.tensor_mul(
                        ycur[0:cout, e0 : e0 + HALO],
                        ycur[0:cout, e0 : e0 + HALO],
                        msk[0:cout, e0 : e0 + HALO],
                    )
                if L < 4:
                    nc.sync.dma_start(ycur[cout : cout + 1, :], ones_row_d[:])

            # ---------------- AllGather y, per-chunk broadcast -------------
            ycc_in = dram.tile([1, CH], f32)
            ycc_out = dram.tile([NCORES, CH], f32)
            nc.sync.dma_start(ycc_in[:], ybuf(4)[0:1, HALO + 1 : HALO + 1 + CH])
            if single:
                for r in range(NCORES):
                    nc.sync.dma_start(ycc_out[r : r + 1, :], ycc_in[:])
            else:
                nc.gpsimd.collective_compute(
                    "AllGather",
                    Op.bypass,
                    replica_groups=rg,
                    ins=[ycc_in[:].opt()],
                    outs=[ycc_out[:].opt()],
                )
            ycc_flat = ycc_out[:].rearrange("a b -> (a b)")
            ychs = []
            for tcn in range(NSUB):
                ych = bigp.tile([128, F], f32, tag="big", bufs=4)
                nc.sync.dma_start(ych[0:1, :], ycc_flat[tcn * F : (tcn + 1) * F])
                nc.gpsimd.partition_broadcast(ych[:], ych[0:1, :])
                ychs.append(ych)

            ones = sb.tile([128, 1], f32)
            nc.vector.memset(ones[:], 1.0)
            shift = sb.tile([128, 1], f32)
            nc.vector.memset(shift[:], SHIFT)

            # ---------------- matvec + per-j-tile softmax/weighted sum -----
            pp = sb.tile([128, JT * NSUB], f32)
            es = sb.tile([128, JT], f32)
            zp = sb.tile([128, JT], f32)
            opp = sb.tile([128, CT * JT], f32)

            def mv(tcn, jt):
                tr = rot.tile([128, F], f32, tag="tr", name=f"tr_{tcn}_{jt}")
                return nc.vector.scalar_tensor_tensor(
                    out=tr[:],
                    in0=wtiles[(tcn, jt)][:],
                    scalar=1.0,
                    in1=ychs[tcn][:],
                    op0=Op.mult,
                    op1=Op.mult,
                    accum_out=pp[:, jt * NSUB + tcn : jt * NSUB + tcn + 1],
                )

            def jt_tail(jt):
                # logits_jt -> exp -> transpose -> broadcast -> weighted sum
                lg = rot.tile([128, 1], f32, tag="lg", name=f"lg{jt}", bufs=2)
                nc.vector.reduce_sum(
                    lg[:],
                    pp[:, jt * NSUB : (jt + 1) * NSUB],
                    axis=Ax.X,
                )
                nc.vector.tensor_add(lg[:], lg[:], blc[:, jt : jt + 1])
                nc.scalar.activation(
                    es[:, jt : jt + 1], lg[:], Act.Exp, bias=shift[:, 0:1],
                    scale=1.0, accum_out=zp[:, jt : jt + 1],
                )
                ept = ztp.tile([16, 128], f32, tag="zt", name=f"ept{jt}", bufs=3)
                nc.tensor.transpose(
                    ept[0:1, :], es[:, jt : jt + 1], ident[:]
                )
                erow = rot.tile([128, 128], f32, tag="erow", name=f"erow{jt}",
                                bufs=2)
                nc.vector.tensor_copy(erow[0:1, :], ept[0:1, :])
                nc.gpsimd.partition_broadcast(erow[:], erow[0:1, :])
                last = None
                for ct in range(CT):
                    trw = rot.tile([128, 128], f32, tag="trw",
                                   name=f"trw{jt}_{ct}", bufs=2)
                    base = ct * W + HALO + 1 + jt * 128
                    last = nc.vector.scalar_tensor_tensor(
                        out=trw[:],
                        in0=xts[:, base : base + 128],
                        scalar=1.0,
                        in1=erow[:],
                        op0=Op.mult,
                        op1=Op.mult,
                        accum_out=opp[:, ct * JT + jt : ct * JT + jt + 1],
                    )
                return last

            for jt in range(JT):
                mv(0, jt)
            prev_tail = None
            for jt in range(JT):
                for tcn in range(1, NSUB):
                    mvi = mv(tcn, jt)
                    if prev_tail is not None and tcn == 1:
                        # keep each j-tile's softmax/weighted-sum tail ahead
                        # of the next j-tile's matvec in the DVE queue, so
                        # tails overlap the Wl stream instead of piling up
                        # at the end
                        add_dep_helper(
                            mvi.ins, prev_tail.ins,
                            reason="interleave jt tail with stream",
                        )
                prev_tail = jt_tail(jt)

            # ---------------- outputs ----------------
            nc.sync.dma_start(out_d[:], opp[:])
            # local softmax denominator: sum zp over free then partitions
            zpr = sb.tile([128, 1], f32)
            nc.vector.reduce_sum(zpr[:], zp[:], axis=Ax.X)
            zps = zpp.tile([1, 1], f32, tag="zps")
            nc.tensor.matmul(zps[:], ones[:], zpr[:], start=True, stop=True)
            zk = sb.tile([1, 1], f32)
            nc.vector.tensor_copy(zk[:], zps[:])
            nc.sync.dma_start(z_d[:], zk[:])

    nc.compile()
    return nc


def _get_nc():
    global _CACHED_NC
    if _CACHED_NC is None:
        _CACHED_NC = _build()
    return _CACHED_NC


def host_prep(**inputs):
    x = np.asarray(inputs["x"], np.float32)
    Wl = np.asarray(inputs["Wl"], np.float32)
    bl = np.asarray(inputs["bl"], np.float32)
    ws = [np.asarray(inputs[f"w{i}"], np.float32) for i in range(1, 6)]
    bss = [np.asarray(inputs[f"b{i}"], np.float32) for i in range(1, 6)]

    xT = np.ascontiguousarray(x.T)  # [DIM, SEQ]

    # packed conv1 weights: w1t[c128, ct*48 + k*16 + o] = w1[o, ct*128+c128, k]
    w1r = np.transpose(ws[0], (1, 2, 0))  # [c, k, o]
    w1t = np.ascontiguousarray(
        w1r.reshape(CT, 128, 48).transpose(1, 0, 2).reshape(128, CT * 48)
    )
    # packed conv2-5 weights + bias row (k=1 slice, row cin)
    wst = np.zeros((17, 45), np.float32)
    for L in range(1, 5):
        cin, cout = CONV[L]
        w = np.transpose(ws[L], (1, 2, 0))  # [cin, k, cout]
        wst[0:cin, WOFF[L] : WOFF[L] + 3 * cout] = w.reshape(cin, -1)
        wst[cin, WOFF[L] + cout : WOFF[L] + 2 * cout] = bss[L]
    bs = np.zeros((16, 1), np.float32)
    bs[:, 0] = bss[0]

    in_maps = []
    for k in range(NCORES):
        s0 = k * CH
        lo, hi = s0 - HALO, s0 + CH + HALO
        xt_k = np.zeros((DIM, EXT), np.float32)
        glo, ghi = max(lo, 0), min(hi, SEQ)
        xt_k[:, glo - lo : ghi - lo] = xT[:, glo:ghi]
        xt_k = np.ascontiguousarray(
            xt_k.reshape(CT, 128, EXT).transpose(1, 0, 2).reshape(128, CT * EXT)
        )
        mask_k = np.zeros((16, W), np.float32)
        tt = np.arange(lo, hi)
        mask_k[:, 1 : W - 1] = ((tt >= 0) & (tt < SEQ)).astype(np.float32)[
            None, :
        ]
        blc_k = np.ascontiguousarray(bl[s0 : s0 + CH].reshape(JT, 128).T)
        wl_k = np.ascontiguousarray(Wl[s0 : s0 + CH, :])
        in_maps.append(
            {
                "xt": xt_k,
                "wl": wl_k,
                "w1t": w1t,
                "wst": wst,
                "bs": bs,
                "blc": blc_k,
                "mask": mask_k,
            }
        )
    return in_maps


def kernel(**inputs):
    global LAST_RESULTS
    in_maps = host_prep(**inputs)
    nc = _get_nc()
    res = run_bass_kernel_spmd(nc, in_maps, core_ids=list(range(NCORES)))
    LAST_RESULTS = res

    total = np.zeros((128, CT * JT), np.float64)
    zsum = 0.0
    for r in res.results:
        total += r["out"]
        zsum += float(r["zout"][0, 0])
    # opp[p, ct*JT + jt] = sum over local t of es * xT; partials summed over
    # cores and j-tiles, then normalized by the global sum of exponentials.
    tot = total.reshape(128, CT, JT).sum(axis=2) / zsum
    return np.ascontiguousarray(tot.T.reshape(DIM)).astype(np.float32)
